# revision 1
# baseline (speedup 1.0000x reference)
"""Trainium2 Bass kernel for nn_KineticEquation (gnn_message_passing).

Reference computation:
    contrib_1 = y[:, i1r] * rate1                 # [B, R1]
    contrib_2 = y[:, i2r0] * y[:, i2r1] * rate2   # [B, R2]
    y_out = scatter_add(contrib_1 -> i1p) + scatter_add(contrib_2 -> i2p)

Strategy (8 cores, full batch per core, second-order reactions sharded by
product tile p//128; each core owns one 128-row slice of y_out^T):

  * y is decomposed host-side as y = yh + yl (two fp8e4m3 terms, ~2^-8
    combined precision); hi/lo occupy the two k-tiles of one fp8 DoubleRow
    matmul (0.5 cyc/row — 2x a bf16 matmul) with the one-hot gather matrix
    duplicated across k-tiles via a stride-0 broadcast access pattern.
  * Second-order reactions are processed in chunks of 128 sharing (T0, T1),
    through two alternating per-chunk pipelines (PSUM has a single read
    port per engine, so no instruction may combine two PSUM operands):
      - Act lane: PE accumulates lnY0+lnY1 gathers into ONE PSUM bank
        (log-space y, fp8 hi/lo), then one Act op computes
        z = exp(g01 + lnRate) with a per-partition bias.
      - DVE lane: separate g0/g1 gathers; DVE stages g0*rate into SBUF
        (tensor_scalar), then multiplies by g1 (one PSUM operand per op).
    z is written as fp8e4m3; chunk pairs share one DoubleRow scatter
    matmul (256-reaction contraction) into the persistent PSUM accumulator.
  * First-order term: dense W1 in fp8 hi/lo, 16 DoubleRow matmuls into the
    same accumulator, scheduled mid-pass.
  * One-hots stream from HBM in groups of GRP chunks to amortize the
    globally-serialized HWDGE descriptor-generation cost per DMA; big PE
    warmup matmuls cover the p-state clock ramp during DMA fill.
"""

import math

import numpy as np

import concourse.tile as tile
from concourse import bacc, mybir
from concourse.bass_utils import run_bass_kernel_spmd

F32 = mybir.dt.float32
BF16 = mybir.dt.bfloat16
F8 = mybir.dt.float8e4
NP_F8 = mybir.dt.np(F8)
NP_BF16 = mybir.dt.np(BF16)

NCORES = 8
P = 128           # partitions / tile edge
S = 1024          # species
NT = S // P       # species tiles (8)
B = 512           # batch
GRP = 16          # chunks per DMA group
MULT = mybir.AluOpType.mult
DR = mybir.MatmulPerfMode.DoubleRow


def _f8_pair(x):
    """Decompose x into hi/lo fp8e4m3 arrays with ~2^-8 combined precision."""
    h = x.astype(NP_F8)
    l = (x - h.astype(np.float32)).astype(NP_F8)
    return h, l


def _preprocess(y_in, i1r, i1p, r1, i2r0, i2r1, i2p, r2):
    """Host-side index preprocessing. Returns per-core input dicts + schedule."""
    i1r = np.asarray(i1r).astype(np.int64)
    i1p = np.asarray(i1p).astype(np.int64)
    i2r0 = np.asarray(i2r0).astype(np.int64)
    i2r1 = np.asarray(i2r1).astype(np.int64)
    i2p = np.asarray(i2p).astype(np.int64)
    r1 = np.asarray(r1).astype(np.float32)
    r2 = np.asarray(r2).astype(np.float32)

    # Dense first-order matrix W1[s, p] = sum of rates
    W1 = np.zeros((S, S), np.float32)
    np.add.at(W1, (i1r, i1p), r1)

    yT = np.ascontiguousarray(np.asarray(y_in, np.float32).T)  # [S, B]
    yh, yl = _f8_pair(yT)
    # [NT, P, 2, B]: hi/lo stacked along the DoubleRow k-tile dim
    yhl = np.ascontiguousarray(
        np.stack([yh.reshape(NT, P, B), yl.reshape(NT, P, B)], axis=2)
    )
    # log-space copy for the Act-engine exp lane: z = exp(lnY0 + lnY1 + lnR)
    lT = np.log(np.maximum(yT, 1e-35))
    lh, ll = _f8_pair(lT)
    lhl = np.ascontiguousarray(
        np.stack([lh.reshape(NT, P, B), ll.reshape(NT, P, B)], axis=2)
    )

    # Shard second-order reactions by product tile
    core_of = i2p >> 7
    T0 = i2r0 >> 7
    T1 = i2r1 >> 7
    binid = (T0 << 3) | T1  # 0..63

    counts = np.zeros((NCORES, NT * NT), np.int64)
    for c in range(NCORES):
        counts[c] = np.bincount(binid[core_of == c], minlength=NT * NT)
    maxc = counts.max(axis=0)
    nch_b = np.ceil(maxc / P).astype(np.int64)          # chunks per bin
    # Order bins by the highest species tile they touch, so the first chunks
    # only need y tiles 0..1 and later tiles can stream in behind them.
    bin_order = sorted(range(NT * NT), key=lambda b: (max(b >> 3, b & 7), b))
    base_b = np.zeros(NT * NT, np.int64)
    acc_chunks = 0
    for b in bin_order:
        base_b[b] = acc_chunks
        acc_chunks += int(nch_b[b])
    nchunk = int(acc_chunks)
    ngroup = math.ceil(nchunk / GRP)
    nchpad = ngroup * GRP

    sched = [None] * nchunk
    for b in bin_order:
        for j in range(int(nch_b[b])):
            sched[base_b[b] + j] = (b >> 3, b & 7)

    in_maps = []
    for c in range(NCORES):
        sel = core_of == c
        bsel = binid[sel]
        order = np.argsort(bsel, kind="stable")
        bs = bsel[order]
        r0l = (i2r0[sel] & 127)[order]
        r1l = (i2r1[sel] & 127)[order]
        pl = (i2p[sel] & 127)[order]
        rr = r2[sel][order]
        bin_start = np.zeros(NT * NT, np.int64)
        cnt = np.bincount(bs, minlength=NT * NT)
        bin_start[1:] = np.cumsum(cnt)[:-1]
        pos = np.arange(len(bs)) - bin_start[bs]
        chunk = base_b[bs] + (pos >> 7)
        col = pos & 127

        G0 = np.zeros((nchpad, P, P), NP_F8)
        G1 = np.zeros((nchpad, P, P), NP_F8)
        # Scatter one-hots in fp8, paired: chunks (2j, 2j+1) occupy the two
        # DoubleRow k-tiles of pair j.
        SC = np.zeros((nchpad // 2, P, 2, P), NP_F8)
        RT = np.zeros((P, nchpad), np.float32)
        G0[chunk, r0l, col] = 1.0
        G1[chunk, r1l, col] = 1.0
        SC[chunk >> 1, col, chunk & 1, pl] = 1.0
        RT[col, chunk] = rr
        LR = np.zeros((P, nchpad), np.float32)
        LR[col, chunk] = np.log(np.maximum(rr, 1e-35))

        def grp2(x):  # [nchpad, P, P] -> [ngroup, P, GRP*P]
            return np.ascontiguousarray(
                x.reshape(ngroup, GRP, P, P).transpose(0, 2, 1, 3).reshape(ngroup, P, GRP * P)
            )

        def grpsc(x):  # [nchpad//2, P, 2, P] -> [ngroup, P, 2, (GRP//2)*P]
            return np.ascontiguousarray(
                x.reshape(ngroup, GRP // 2, P, 2, P)
                .transpose(0, 2, 3, 1, 4)
                .reshape(ngroup, P, 2, (GRP // 2) * P)
            )

        # First-order matrix as fp8 hi/lo: [NT, P, 2, P] indexed [t, s, hl, p]
        w1h, w1l = _f8_pair(W1[:, c * P:(c + 1) * P])
        w1f = np.ascontiguousarray(
            np.stack([w1h.reshape(NT, P, P), w1l.reshape(NT, P, P)], axis=2)
        )

        ctl = np.ascontiguousarray(np.concatenate([RT, LR], axis=1))
        in_maps.append(
            dict(
                YHL=yhl,
                LHL=lhl,
                W1F=w1f,
                G0=grp2(G0),
                G1=grp2(G1),
                SCT=grpsc(SC),
                CTL=ctl,
            )
        )
    return in_maps, sched, nchunk, ngroup


def _build(nchunk, ngroup, sched, reps=1, bufs_oh=4, bufs_g0=2, bufs_g1=2,
           lag=6, warmup=5, npre=3, dve_share=(1, 2)):
    nc = bacc.Bacc("TRN2", target_bir_lowering=False, debug=False, num_devices=NCORES)
    nchpad = ngroup * GRP

    yhl_d = nc.dram_tensor("YHL", [NT, P, 2, B], F8, kind="ExternalInput").ap()
    lhl_d = nc.dram_tensor("LHL", [NT, P, 2, B], F8, kind="ExternalInput").ap()
    w1_d = nc.dram_tensor("W1F", [NT, P, 2, P], F8, kind="ExternalInput").ap()
    g0_d = nc.dram_tensor("G0", [ngroup, P, GRP * P], F8, kind="ExternalInput").ap()
    g1_d = nc.dram_tensor("G1", [ngroup, P, GRP * P], F8, kind="ExternalInput").ap()
    sc_d = nc.dram_tensor("SCT", [ngroup, P, 2, (GRP // 2) * P], F8, kind="ExternalInput").ap()
    ctl_d = nc.dram_tensor("CTL", [P, 2 * nchpad], F32, kind="ExternalInput").ap()
    out_d = nc.dram_tensor("out", [P, B], F32, kind="ExternalOutput").ap()

    with tile.TileContext(nc) as tc:
        with (
            tc.tile_pool(name="res", bufs=1) as res,
            tc.tile_pool(name="oh", bufs=bufs_oh) as ohp,
            tc.tile_pool(name="zp", bufs=lag + 2) as zp,
            tc.tile_pool(name="acc", bufs=1, space="PSUM") as accp,
            tc.tile_pool(name="gp0", bufs=bufs_g0, space="PSUM") as gp0p,
            tc.tile_pool(name="gp1", bufs=bufs_g1, space="PSUM") as gp1p,
            tc.tile_pool(name="gpa", bufs=3, space="PSUM") as gpap,
        ):
            # PE warmup: big dependency-free matmuls spanning the initial DMA
            # window so the p-state clock ramp (3us to full speed) completes
            # before the first real matmul.
            if warmup:
                wt = res.tile([P, B], BF16, tag="warm")
                nc.vector.memset(wt[:], 0.0)
                wps = accp.tile([P, B], F32, space="PSUM", tag="acc")
                for _ in range(warmup):
                    nc.tensor.matmul(wps[:], lhsT=wt[:, :P], rhs=wt[:],
                                     start=True, stop=True)

            # Early y/ln tiles individually (fine-grained arrival); late
            # tiles merged — each dma_start costs ~0.5-0.8us of globally
            # serialized HWDGE issue, so DMA count is precious.
            yhl = [res.tile([P, 2, B], F8, tag=f"yhl{t}", name=f"yhl{t}") for t in range(4)]
            lhl = [res.tile([P, 2, B], F8, tag=f"lhl{t}", name=f"lhl{t}") for t in range(4)]
            ylate = res.tile([P, 4, 2, B], F8, tag="ylate")
            llate = res.tile([P, 4, 2, B], F8, tag="llate")
            yv = [yhl[t][:] for t in range(4)] + [ylate[:, t, :, :] for t in range(4)]
            lv = [lhl[t][:] for t in range(4)] + [llate[:, t, :, :] for t in range(4)]
            for t in range(2):
                nc.sync.dma_start(yhl[t][:], yhl_d[t])
            ctl = res.tile([P, 2 * nchpad], F32, tag="ctl")
            nc.sync.dma_start(ctl[:], ctl_d[:])
            rt = ctl[:, 0:nchpad]
            lr = ctl[:, nchpad:2 * nchpad]
            for t in range(2):
                nc.sync.dma_start(lhl[t][:], lhl_d[t])

            pre = []
            for gi in range(min(npre, ngroup)):
                pg0 = ohp.tile([P, 1, GRP * P], F8, tag="g0g")
                pg1 = ohp.tile([P, 1, GRP * P], F8, tag="g1g")
                psc = ohp.tile([P, 2, (GRP // 2) * P], F8, tag="scg")
                nc.sync.dma_start(pg0[:, 0, :], g0_d[gi])
                nc.sync.dma_start(pg1[:, 0, :], g1_d[gi])
                nc.sync.dma_start(psc[:], sc_d[gi])
                pre.append((pg0, pg1, psc))
                if gi == 1:
                    for t in range(2, 4):
                        nc.sync.dma_start(yhl[t][:], yhl_d[t])
                        nc.sync.dma_start(lhl[t][:], lhl_d[t])
            nc.sync.dma_start(ylate[:], yhl_d[4:].transpose((1, 0, 2, 3)))
            nc.sync.dma_start(llate[:], lhl_d[4:].transpose((1, 0, 2, 3)))
            w1t = res.tile([P, NT, 2, P], F8, tag="w1")
            nc.sync.dma_start(w1t[:], w1_d.transpose((1, 0, 2, 3)))

            num, den = dve_share

            def lane_dve(c):
                return ((c // 2) * 5) % 16 < 5

            def dr_mm(out_ps, oh_tile, cs, t, start=True, stop=True, src=None):
                lhsT = oh_tile[:, :, cs].broadcast_to([P, 2, P])
                nc.tensor.matmul(out_ps, lhsT=lhsT,
                                 rhs=(src if src is not None else yv)[t],
                                 start=start, stop=stop, perf_mode=DR)

            def one_pass():
                acc = accp.tile([P, B], F32, space="PSUM", tag="acc")
                first_acc = [True]

                def acc_mm(lhsT, rhs, stop=False, perf_mode=None):
                    nc.tensor.matmul(acc[:], lhsT=lhsT, rhs=rhs,
                                     start=first_acc[0], stop=stop,
                                     perf_mode=perf_mode)
                    first_acc[0] = False

                # Second-order chunks, software-pipelined by `lag` chunks.
                from collections import deque
                pending = deque()
                # First-order W1 matmuls, spread one per chunk after the w1t
                # operand has landed, to avoid an inline PE-queue block.
                dense_list = [(t, hl) for t in range(NT) for hl in range(2)]
                dense_start = 48

                tiles = {}
                for gi in range(min(npre, ngroup)) if _rep_is_first[0] else []:
                    tiles[gi] = pre[gi]

                def fetch(gi):
                    if gi in tiles or gi >= ngroup:
                        return
                    g0g = ohp.tile([P, 1, GRP * P], F8, tag="g0g")
                    g1g = ohp.tile([P, 1, GRP * P], F8, tag="g1g")
                    scg = ohp.tile([P, 2, (GRP // 2) * P], F8, tag="scg")
                    nc.sync.dma_start(g0g[:, 0, :], g0_d[gi])
                    nc.sync.dma_start(g1g[:, 0, :], g1_d[gi])
                    nc.sync.dma_start(scg[:], sc_d[gi])
                    tiles[gi] = (g0g, g1g, scg)

                for c in range(nchunk):
                    t0, t1 = sched[c]
                    gi, k = divmod(c, GRP)
                    fetch(gi)
                    if k == GRP - 12:
                        fetch(gi + 1)
                    g0g, g1g, scg = tiles[gi]
                    cs = slice(k * P, (k + 1) * P)

                    if c % 2 == 0:
                        zpair = zp.tile([P, 2, B], F8, tag="z")
                        if c == nchunk - 1:
                            # odd chunk count: zero the unused k-tile so the
                            # paired scatter never multiplies garbage
                            nc.vector.memset(zpair[:, 1, :], 0.0)
                    if lane_dve(c):
                        # DVE lane: two gathers; PSUM has a single DVE read
                        # port, so stage g0*rate into SBUF first, then
                        # multiply by g1 (one PSUM operand per instruction).
                        g0p = gp0p.tile([P, B], F32, space="PSUM", tag="g0p")
                        g1p = gp1p.tile([P, B], F32, space="PSUM", tag="g1p")
                        dr_mm(g0p[:], g0g, cs, t0)
                        dr_mm(g1p[:], g1g, cs, t1)
                        g0s = zp.tile([P, B], BF16, tag="g0s")
                        nc.vector.tensor_scalar(
                            out=g0s[:], in0=g0p[:], scalar1=rt[:, c:c + 1],
                            scalar2=None, op0=MULT,
                        )
                        nc.vector.tensor_tensor(
                            out=zpair[:, c % 2, :], in0=g0s[:], in1=g1p[:],
                            op=MULT,
                        )
                    else:
                        # Act lane: log-space gathers accumulate into one PSUM
                        # bank; z = exp(lnY0 + lnY1 + lnR) on the Act engine
                        g01 = gpap.tile([P, B], F32, space="PSUM", tag="g01")
                        dr_mm(g01[:], g0g, cs, t0, start=True, stop=False, src=lv)
                        dr_mm(g01[:], g1g, cs, t1, start=False, stop=True, src=lv)
                        nc.scalar.activation(
                            zpair[:, c % 2, :], g01[:],
                            mybir.ActivationFunctionType.Exp,
                            bias=lr[:, c:c + 1], scale=1.0,
                        )

                    if c % 2 == 1 or c == nchunk - 1:
                        j = (c // 2) % (GRP // 2)
                        ps = slice(j * P, (j + 1) * P)
                        pending.append((scg[:, :, ps], zpair))
                    di = c - dense_start
                    if 0 <= di < len(dense_list):
                        t, hl = dense_list[di]
                        lhsT = w1t[:, t, hl:hl + 1, :].broadcast_to([P, 2, P])
                        nc.tensor.matmul(
                            acc[:], lhsT=lhsT, rhs=yv[t],
                            start=first_acc[0], stop=False, perf_mode=DR)
                        first_acc[0] = False
                    if len(pending) > lag:
                        lh, zz = pending.popleft()
                        nc.tensor.matmul(acc[:], lhsT=lh, rhs=zz[:],
                                         start=first_acc[0], stop=False,
                                         perf_mode=DR)
                        first_acc[0] = False

                while pending:
                    lh, zz = pending.popleft()
                    nc.tensor.matmul(acc[:], lhsT=lh, rhs=zz[:],
                                     start=first_acc[0], stop=(len(pending) == 0),
                                     perf_mode=DR)
                    first_acc[0] = False

                outs = zp.tile([P, B], F32, tag="outs")
                nc.vector.tensor_copy(outs[:], acc[:])
                nc.sync.dma_start(out_d[:], outs[:])

            _rep_is_first = [True]
            for _rep in range(reps):
                one_pass()
                _rep_is_first[0] = False

    nc.compile()
    return nc


def _run(inputs, trace=False):
    in_maps, sched, nchunk, ngroup = _preprocess(
        inputs["y_in"], inputs["inds_1r"], inputs["inds_1p"], inputs["rate_1"],
        inputs["inds_2r0"], inputs["inds_2r1"], inputs["inds_2p"], inputs["rate_2"],
    )
    nc = _build(nchunk, ngroup, sched)
    res = None
    y_out = None
    last_exc = None
    for attempt in range(3):
        try:
            res = run_bass_kernel_spmd(nc, in_maps, list(range(NCORES)), trace=trace)
        except Exception as e:  # transient device wedges
            last_exc = e
            import time as _time
            _time.sleep(2.0)
            continue
        y_out = np.empty((B, S), np.float32)
        for c in range(NCORES):
            y_out[:, c * P:(c + 1) * P] = res.results[c]["out"].T
        if np.isfinite(y_out).all() and not (y_out == 0).all():
            break
        y_out = None
    if y_out is None:
        if last_exc is not None:
            raise last_exc
        raise RuntimeError("kernel produced non-finite/empty output on all attempts")
    return y_out, res


def kernel(**inputs) -> np.ndarray:
    return _run(inputs, trace=False)[0]

